# revision 2
# baseline (speedup 1.0000x reference)
"""Causal single-head attention (4096x2048, d=128) on 8 TRN2 NeuronCores.

Strategy (flash-style sequence parallelism):
- Q rows are sharded mod-8 across cores: core c owns global rows c::8.
  With causal masking this gives every core an *identical* work profile,
  so one SPMD program needs no per-core control flow.
- K/V projections are sharded by contiguous 512-key blocks; the projected
  K^T and V (bf16, 1MB each) are AllGathered instead of replicating the
  64MB fp32 K/V input streams on every core.
- Scores are computed transposed ([keys, rows] = K^T tiles as stationary,
  Q^T as moving) so exp(P) tiles feed the P@V matmul directly as the
  stationary operand with no on-chip transposes of P.
- The softmax denominator is obtained by appending a ones-column to V:
  out_psum[:, 0] accumulates sum_s P[s, r] alongside P@V.
- Causal mask is multiplicative (0/1) after exp, applied only to the 8
  diagonal-band key tiles per query tile; the mask pattern is independent
  of the query-tile index and is passed per-core from the host.
"""

import math
import sys

sys.path.insert(0, "/opt/trn_rl_repo")

import ml_dtypes
import numpy as np

import concourse.bass as bass
import concourse.tile as tile
from concourse import bacc, mybir
from concourse.bass import ts
from concourse.bass_utils import run_bass_kernel_spmd
from concourse.masks import make_identity

N_CORES = 8
SEQ = 4096
D_MODEL = 2048
D_HEAD = 128
R = SEQ // N_CORES          # 512 query rows per core
KB = SEQ // N_CORES         # 512 keys projected per core
N_QT = R // 128             # 4 query tiles of 128 rows per core
N_MT = D_MODEL // 128       # 16 contraction tiles for projections
N_KT = SEQ // 128           # 32 key tiles total
INV_SQRT = 1.0 / math.sqrt(D_MODEL)
VSTRIDE = 130               # V tile stride in Vaug (1 ones-col + 128 dims + pad)

BF16 = mybir.dt.bfloat16
F32 = mybir.dt.float32


def _build():
    nc = bacc.Bacc("TRN2", target_bir_lowering=False, debug=False,
                   num_devices=N_CORES)

    iq_t = nc.dram_tensor("iq_t", [D_MODEL, R], BF16, kind="ExternalInput").ap()
    ik_t = nc.dram_tensor("ik_t", [D_MODEL, KB], BF16, kind="ExternalInput").ap()
    iv_t = nc.dram_tensor("iv_t", [D_MODEL, KB], BF16, kind="ExternalInput").ap()
    wq_t = nc.dram_tensor("wq_t", [D_MODEL, D_HEAD], BF16, kind="ExternalInput").ap()
    wk_t = nc.dram_tensor("wk_t", [D_MODEL, D_HEAD], BF16, kind="ExternalInput").ap()
    wv_t = nc.dram_tensor("wv_t", [D_MODEL, D_HEAD], BF16, kind="ExternalInput").ap()
    bq = nc.dram_tensor("bq", [D_HEAD, 1], F32, kind="ExternalInput").ap()
    bk = nc.dram_tensor("bk", [D_HEAD, 1], F32, kind="ExternalInput").ap()
    bv = nc.dram_tensor("bv", [D_HEAD, 1], F32, kind="ExternalInput").ap()
    mask = nc.dram_tensor("mask", [128, 8 * 128], BF16, kind="ExternalInput").ap()
    out = nc.dram_tensor("out", [R, D_HEAD], F32, kind="ExternalOutput").ap()

    with tile.TileContext(nc) as tc:
        with (
            tc.tile_pool(name="const", bufs=1) as const,
            tc.tile_pool(name="inbuf", bufs=6) as inbuf,
            tc.tile_pool(name="work", bufs=3) as work,
            tc.tile_pool(name="psum", bufs=2, space="PSUM") as psum,
            tc.tile_pool(name="dram", bufs=1, space="DRAM") as dram,
        ):
            # ---- constants ----
            ident = const.tile([128, 128], BF16)
            make_identity(nc, ident)
            bq_sb = const.tile([D_HEAD, 1], F32)
            bk_sb = const.tile([D_HEAD, 1], F32)
            bv_sb = const.tile([D_HEAD, 1], F32)
            nc.sync.dma_start(bq_sb[:], bq[:])
            nc.sync.dma_start(bk_sb[:], bk[:])
            nc.sync.dma_start(bv_sb[:], bv[:])
            mask_sb = const.tile([128, 8 * 128], BF16)
            nc.sync.dma_start(mask_sb[:], mask[:])
            # weights: [128, 16*128], m-tile t at cols [128t, 128t+128)
            w_sbs = {}
            for name, wdram in (("wq", wq_t), ("wk", wk_t), ("wv", wv_t)):
                w_sb = const.tile([128, D_MODEL], BF16, name=f"{name}_sb")
                for t in range(N_MT):
                    nc.sync.dma_start(w_sb[:, ts(t, 128)], wdram[ts(t, 128), :])
                w_sbs[name] = w_sb

            # ---- projections: out^T = W @ x^T, accumulated over 16 m-tiles --
            def project(w_sb, x_dram, ncols, bias_sb, out_sb):
                xt = []
                for t in range(N_MT):
                    x_in = inbuf.tile([128, ncols], BF16, name="x_in", tag="xin")
                    nc.sync.dma_start(x_in[:], x_dram[ts(t, 128), :])
                    xt.append(x_in)
                pp = psum.tile([128, ncols], F32, name="proj_ps", tag="proj")
                for t in range(N_MT):
                    nc.tensor.matmul(pp[:], w_sb[:, ts(t, 128)], xt[t][:],
                                     start=(t == 0), stop=(t == N_MT - 1))
                # PSUM -> SBUF bf16 with per-partition bias add
                nc.vector.tensor_scalar_add(out_sb[:], pp[:], bias_sb[:])

            kt_sb = work.tile([128, KB], BF16, bufs=1)     # K^T local block
            vt_sb = work.tile([128, KB], BF16, bufs=1)     # V^T local block
            qt_sb = work.tile([128, R], BF16, bufs=1)      # Q^T (kept resident)
            project(w_sbs["wk"], ik_t, KB, bk_sb, kt_sb)
            project(w_sbs["wv"], iv_t, KB, bv_sb, vt_sb)
            project(w_sbs["wq"], iq_t, R, bq_sb, qt_sb)

            # ---- AllGather K^T ----
            cc_k_in = dram.tile([128, KB], BF16)
            cc_k_out = dram.tile([N_CORES, 128, KB], BF16, addr_space="Shared")
            nc.sync.dma_start(cc_k_in[:], kt_sb[:])
            nc.gpsimd.collective_compute(
                "AllGather", mybir.AluOpType.bypass,
                replica_groups=[list(range(N_CORES))],
                ins=[cc_k_in.opt()], outs=[cc_k_out.opt()],
            )

            # ---- transpose V^T -> V block rows, AllGather V ----
            vtr_sb = work.tile([128, KB], BF16, bufs=1)    # V rows (s, d) x4
            for t in range(KB // 128):
                tp = psum.tile([128, 128], BF16, name="tp_ps", tag="tp")
                nc.tensor.transpose(tp[:], vt_sb[:, ts(t, 128)], ident[:])
                nc.vector.tensor_copy(vtr_sb[:, ts(t, 128)], tp[:])
            cc_v_in = dram.tile([KB, 128], BF16)
            cc_v_out = dram.tile([N_CORES, KB, 128], BF16, addr_space="Shared")
            for t in range(KB // 128):
                nc.sync.dma_start(cc_v_in[ts(t, 128), :], vtr_sb[:, ts(t, 128)])
            nc.gpsimd.collective_compute(
                "AllGather", mybir.AluOpType.bypass,
                replica_groups=[list(range(N_CORES))],
                ins=[cc_v_in.opt()], outs=[cc_v_out.opt()],
            )

            # ---- load gathered K^T [128, 4096] and Vaug [128, 32*130] ----
            ktf_sb = const.tile([128, SEQ], BF16)
            for r in range(N_CORES):
                nc.sync.dma_start(ktf_sb[:, ts(r, KB)], cc_k_out[r])
            vaug_sb = const.tile([128, N_KT * VSTRIDE], BF16)
            va = vaug_sb.rearrange("p (t x) -> p t x", t=N_KT)
            nc.vector.memset(vaug_sb[:], 1.0)
            cc_v_flat = cc_v_out.rearrange("r (t p) d -> (r t) p d", t=KB // 128)
            for t in range(N_KT):
                nc.sync.dma_start(va[:, t, 1:129], cc_v_flat[t])

            # ---- attention: per query tile j, key groups of 4 tiles ----
            for j in range(N_QT):
                n_groups = 2 * (j + 1)
                opsum = psum.tile([128, 129], F32, name="opsum", tag="acc")
                for g in range(n_groups):
                    spsum = psum.tile([128, 512], F32, name="spsum", tag="sc")
                    for q in range(4):
                        kt = 4 * g + q
                        nc.tensor.matmul(
                            spsum[:, ts(q, 128)],
                            ktf_sb[:, ts(kt, 128)],
                            qt_sb[:, ts(j, 128)],
                            start=True, stop=True,
                        )
                    p_sb = work.tile([128, 512], BF16, name="p_sb", tag="p",
                                     bufs=3)
                    nc.scalar.activation(p_sb[:], spsum[:],
                                         mybir.ActivationFunctionType.Exp,
                                         scale=INV_SQRT)
                    if g >= 2 * j:
                        nc.vector.tensor_mul(p_sb[:], p_sb[:],
                                             mask_sb[:, ts(g - 2 * j, 512)])
                    for q in range(4):
                        kt = 4 * g + q
                        nc.tensor.matmul(
                            opsum[:],
                            p_sb[:, ts(q, 128)],
                            va[:, kt, 0:129],
                            start=(g == 0 and q == 0),
                            stop=(g == n_groups - 1 and q == 3),
                        )
                recip = work.tile([128, 1], F32, name="recip", tag="recip")
                nc.vector.reciprocal(recip[:], opsum[:, 0:1])
                o_sb = work.tile([128, D_HEAD], F32, name="o_sb", tag="o")
                nc.vector.tensor_scalar_mul(o_sb[:], opsum[:, 1:129], recip[:])
                nc.sync.dma_start(out[ts(j, 128), :], o_sb[:])

    nc.compile()
    return nc


_NC_CACHE = None


def _get_nc():
    global _NC_CACHE
    if _NC_CACHE is None:
        _NC_CACHE = _build()
    return _NC_CACHE


def make_in_maps(input_q, input_k, input_v, WQ_w, WQ_b, WK_w, WK_b, WV_w, WV_b):
    bf16 = ml_dtypes.bfloat16
    input_q = np.asarray(input_q, dtype=np.float32)
    input_k = np.asarray(input_k, dtype=np.float32)
    input_v = np.asarray(input_v, dtype=np.float32)
    wq_t = np.ascontiguousarray(np.asarray(WQ_w, np.float32).T).astype(bf16)
    wk_t = np.ascontiguousarray(np.asarray(WK_w, np.float32).T).astype(bf16)
    wv_t = np.ascontiguousarray(np.asarray(WV_w, np.float32).T).astype(bf16)
    bq_h = np.asarray(WQ_b, np.float32).reshape(D_HEAD, 1)
    bk_h = np.asarray(WK_b, np.float32).reshape(D_HEAD, 1)
    bv_h = np.asarray(WV_b, np.float32).reshape(D_HEAD, 1)

    s = np.arange(128)[:, None, None]     # s_local (key within tile)
    m = np.arange(8)[None, :, None]       # diag-band key tile index
    r = np.arange(128)[None, None, :]     # r_local (query within tile)
    in_maps = []
    for c in range(N_CORES):
        mask_c = np.ascontiguousarray(
            ((128 * m + s) <= (8 * r + c)).transpose(0, 1, 2)
        ).astype(bf16).reshape(128, 1024)
        in_maps.append({
            "iq_t": np.ascontiguousarray(input_q[c::8].T).astype(bf16),
            "ik_t": np.ascontiguousarray(input_k[KB * c:KB * (c + 1)].T).astype(bf16),
            "iv_t": np.ascontiguousarray(input_v[KB * c:KB * (c + 1)].T).astype(bf16),
            "wq_t": wq_t, "wk_t": wk_t, "wv_t": wv_t,
            "bq": bq_h, "bk": bk_h, "bv": bv_h,
            "mask": mask_c,
        })
    return in_maps


def assemble(results):
    full = np.empty((SEQ, D_HEAD), dtype=np.float32)
    for c in range(N_CORES):
        full[c::8] = results[c]["out"]
    return full


def kernel(**inputs):
    nc = _get_nc()
    in_maps = make_in_maps(**inputs)
    res = run_bass_kernel_spmd(nc, in_maps, core_ids=list(range(N_CORES)))
    return assemble(res.results)
